# revision 12
# baseline (speedup 1.0000x reference)
"""Trainium2 Bass kernel for BiDecoder edge dot products.

out[e] = dot(ufeat[src[e]], ifeat[dst[e]])   for E=300000 edges, D=256.

Strategy (8 NeuronCores, SPMD):
  - Shard edges across the 8 cores (37500 each); replicate ufeat (fp16),
    and give each core a host-compacted ifeat table holding only its
    distinct dst rows (~26.4k < 32767, one int16 gather base).
  - Edges are dst-sorted, so compacted dst rows form a dense
    non-decreasing sequence; greedy segmentation packs runs of 4/2/1
    consecutive rows into quad (2KB) / pair (1KB) / single (512B) gather
    descriptors (~53%/32%/16% of edges), cutting DMA packet overhead
    (measured ~20ns fixed + ~16ns per 512B per packet) and SWDGE
    descriptor-generation entries.
  - Quad chunks (512 quads = 2048 edges: 1 hv gather + 2 hu gathers),
    pair chunks (1024 pairs = 2048 edges: 1+2) and single chunks (1024
    edges: 1+1) run through separate slot pools on an interleaved
    schedule; 4 SWDGE queues rotate per call.
  - DVE: tensor_tensor(mult) product (2-byte packed -> 2x mode), a
    2x-mode fold of k vs k+128, then a half-width tensor_reduce(axis=X);
    one final DMA writes the dots. Host reorders to original edge order.
"""

import sys

for _p in ("/opt/trn_rl_repo",):
    if _p not in sys.path:
        sys.path.append(_p)

import numpy as np

F16 = np.float16

P = 128
D = 256
E = 300000
NCORES = 8
ECORE = E // NCORES
N_GENE = 20000
N_CELL = 50000
CHUNK_E = 1024           # idx entries per pair/single dma_gather call
QCHUNK = 512             # idx entries per quad call (2048 edges)
COLS = CHUNK_E // 16
QCOLS = QCHUNK // 16
NSLOT_Q = 4
NSLOT_P = 3
NSLOT_S = 3

_PROGRAM_CACHE: dict = {}


def _cdiv(a, b):
    return -(-a // b)


def _wrap_idx(idx_i16: np.ndarray, ncall: int, entries: int) -> np.ndarray:
    """[ncall*entries] int16 -> [128, ncall*entries//16] dma_gather idx layout."""
    cols = entries // 16
    w = idx_i16.reshape(ncall, cols, 16).transpose(2, 0, 1).reshape(16, ncall * cols)
    return np.ascontiguousarray(np.tile(w, (8, 1)))


def _build_program(nqc: int, npc: int, nsc: int, vcap: int, n_gene: int = N_GENE):
    import concourse.bacc as bacc
    import concourse.bass as bassmod
    import concourse.mybir as mybir
    from concourse.library_config import mlp

    nqc_t, npc_t, nsc_t = max(1, nqc), max(1, npc), max(1, nsc)
    ycols = nqc * 16 + npc * 16 + nsc_t * 8

    nc = bacc.Bacc("TRN2", debug=False, num_swdge_queues=4,
                   dynamic_dma_scratch_size=65536)
    ufeat = nc.dram_tensor("ufeat", [n_gene, D], mybir.dt.float16, kind="ExternalInput")
    # 3 pad rows so the overlapping 4-row quad AP stays in bounds
    vtab = nc.dram_tensor("vtab", [vcap + 3, D], mybir.dt.float16, kind="ExternalInput")
    sidxq = nc.dram_tensor("sidxq", [P, nqc_t * 2 * COLS], mybir.dt.int16, kind="ExternalInput")
    didxq = nc.dram_tensor("didxq", [P, nqc_t * QCOLS], mybir.dt.int16, kind="ExternalInput")
    sidxp = nc.dram_tensor("sidxp", [P, npc_t * 2 * COLS], mybir.dt.int16, kind="ExternalInput")
    didxp = nc.dram_tensor("didxp", [P, npc_t * COLS], mybir.dt.int16, kind="ExternalInput")
    sidxs = nc.dram_tensor("sidxs", [P, nsc_t * COLS], mybir.dt.int16, kind="ExternalInput")
    didxs = nc.dram_tensor("didxs", [P, nsc_t * COLS], mybir.dt.int16, kind="ExternalInput")
    y = nc.dram_tensor("y", [P, ycols], mybir.dt.float32, kind="ExternalOutput")

    with (
        nc.sbuf_tensor("huq", [P, NSLOT_Q, 2, 8, D], mybir.dt.float16) as huq,
        nc.sbuf_tensor("hvq", [P, NSLOT_Q, 16, D], mybir.dt.float16) as hvq,
        nc.sbuf_tensor("hup", [P, NSLOT_P, 2, 8, D], mybir.dt.float16) as hup,
        nc.sbuf_tensor("hvp", [P, NSLOT_P, 16, D], mybir.dt.float16) as hvp,
        nc.sbuf_tensor("hus", [P, NSLOT_S, 8, D], mybir.dt.float16) as hus,
        nc.sbuf_tensor("hvs", [P, NSLOT_S, 8, D], mybir.dt.float16) as hvs,
        nc.sbuf_tensor("sidxq_sb", [P, nqc_t * 2 * COLS], mybir.dt.int16) as sidxq_sb,
        nc.sbuf_tensor("didxq_sb", [P, nqc_t * QCOLS], mybir.dt.int16) as didxq_sb,
        nc.sbuf_tensor("sidxp_sb", [P, npc_t * 2 * COLS], mybir.dt.int16) as sidxp_sb,
        nc.sbuf_tensor("didxp_sb", [P, npc_t * COLS], mybir.dt.int16) as didxp_sb,
        nc.sbuf_tensor("sidxs_sb", [P, nsc_t * COLS], mybir.dt.int16) as sidxs_sb,
        nc.sbuf_tensor("didxs_sb", [P, nsc_t * COLS], mybir.dt.int16) as didxs_sb,
        nc.sbuf_tensor("osb", [P, ycols], mybir.dt.float32) as osb,
        nc.semaphore("io") as io,
        nc.semaphore("cons") as cons,
        nc.semaphore("io2") as io2,
        nc.Block(no_gpsimd_drain=True) as block,
        __import__("contextlib").ExitStack() as _stk,
    ):
        gq_sem = [_stk.enter_context(nc.semaphore(f"gq{i}")) for i in range(NSLOT_Q)]
        gp_sem = [_stk.enter_context(nc.semaphore(f"gp{i}")) for i in range(NSLOT_P)]
        gs_sem = [_stk.enter_context(nc.semaphore(f"gs{i}")) for i in range(NSLOT_S)]

        vtab_pair_ap = bassmod.AP(vtab, 0, [[D, vcap], [1, 2 * D]])
        vtab_quad_ap = bassmod.AP(vtab, 0, [[D, vcap], [1, 4 * D]])

        # fair interleave of the three chunk types
        schedule = []
        rem = {"Q": nqc, "P": npc, "S": nsc}
        done = {"Q": 0, "P": 0, "S": 0}
        total = nqc + npc + nsc
        for _ in range(total):
            kind = max(rem, key=lambda t: rem[t] - done[t] * 0)  # placeholder
            # pick the type furthest behind its proportional pace
            best, bestv = None, -1e9
            for t in ("Q", "P", "S"):
                if done[t] < rem[t]:
                    v = rem[t] / total * (len(schedule) + 1) - done[t]
                    if v > bestv:
                        best, bestv = t, v
            schedule.append((best, done[best]))
            done[best] += 1
        gidx = {"Q": {}, "P": {}, "S": {}}
        for g, (kind, c) in enumerate(schedule):
            gidx[kind][c] = g

        @block.gpsimd
        def _(gp):
            gp.load_library(mlp)
            q = 0
            started = {"Q": False, "P": False, "S": False}
            gate = {"Q": 32, "P": 64, "S": 96}
            nslt = {"Q": NSLOT_Q, "P": NSLOT_P, "S": NSLOT_S}
            for kind, c in schedule:
                if not started[kind]:
                    gp.wait_ge(io, gate[kind])
                    started[kind] = True
                ns = nslt[kind]
                s = c % ns
                if c >= ns:
                    gp.wait_ge(cons, gidx[kind][c - ns] + 1)
                if kind == "Q":
                    gp.dma_gather(
                        hvq[:, s].rearrange("p (a b) k -> p a (b k)", b=4),
                        vtab_quad_ap,
                        didxq_sb[:, c * QCOLS : (c + 1) * QCOLS],
                        QCHUNK, QCHUNK, 4 * D, elem_step=D,
                        queue_num=q % 4, single_packet=False,
                    ).then_inc(gq_sem[s], 16)
                    for h in range(2):
                        cols = slice((2 * c + h) * COLS, (2 * c + h + 1) * COLS)
                        gp.dma_gather(
                            huq[:, s, h], ufeat[:, :], sidxq_sb[:, cols],
                            CHUNK_E, CHUNK_E, D,
                            queue_num=(q + 1 + h) % 4, single_packet=False,
                        ).then_inc(gq_sem[s], 16)
                    q += 3
                elif kind == "P":
                    gp.dma_gather(
                        hvp[:, s].rearrange("p (a b) k -> p a (b k)", b=2),
                        vtab_pair_ap,
                        didxp_sb[:, c * COLS : (c + 1) * COLS],
                        CHUNK_E, CHUNK_E, 2 * D, elem_step=D,
                        queue_num=q % 4, single_packet=False,
                    ).then_inc(gp_sem[s], 16)
                    for h in range(2):
                        cols = slice((2 * c + h) * COLS, (2 * c + h + 1) * COLS)
                        gp.dma_gather(
                            hup[:, s, h], ufeat[:, :], sidxp_sb[:, cols],
                            CHUNK_E, CHUNK_E, D,
                            queue_num=(q + 1 + h) % 4, single_packet=False,
                        ).then_inc(gp_sem[s], 16)
                    q += 3
                else:
                    cols = slice(c * COLS, (c + 1) * COLS)
                    gp.dma_gather(
                        hus[:, s], ufeat[:, :], sidxs_sb[:, cols],
                        CHUNK_E, CHUNK_E, D,
                        queue_num=q % 4, single_packet=False,
                    ).then_inc(gs_sem[s], 16)
                    gp.dma_gather(
                        hvs[:, s], vtab[: vcap + 3, :], didxs_sb[:, cols],
                        CHUNK_E, CHUNK_E, D,
                        queue_num=(q + 1) % 4, single_packet=False,
                    ).then_inc(gs_sem[s], 16)
                    q += 2
            for s in range(NSLOT_Q):
                cnt = (nqc - s + NSLOT_Q - 1) // NSLOT_Q
                if cnt > 0:
                    gp.wait_ge(gq_sem[s], 48 * cnt)
            for s in range(NSLOT_P):
                cnt = (npc - s + NSLOT_P - 1) // NSLOT_P
                if cnt > 0:
                    gp.wait_ge(gp_sem[s], 48 * cnt)
            for s in range(NSLOT_S):
                cnt = (nsc - s + NSLOT_S - 1) // NSLOT_S
                if cnt > 0:
                    gp.wait_ge(gs_sem[s], 32 * cnt)

        @block.vector
        def _(v):
            H = D // 2
            for kind, c in schedule:
                if kind == "Q":
                    s = c % NSLOT_Q
                    k = c // NSLOT_Q + 1
                    v.wait_ge(gq_sem[s], 48 * k)
                    for h in range(2):
                        v.tensor_tensor(
                            out=hvq[:, s, 8 * h : 8 * h + 8, :],
                            in0=huq[:, s, h],
                            in1=hvq[:, s, 8 * h : 8 * h + 8, :],
                            op=mybir.AluOpType.mult,
                        )
                    v.tensor_tensor(
                        out=hvq[:, s, :, 0:H],
                        in0=hvq[:, s, :, 0:H],
                        in1=hvq[:, s, :, H:D],
                        op=mybir.AluOpType.add,
                    )
                    v.tensor_reduce(
                        out=osb[:, c * 16 : (c + 1) * 16],
                        in_=hvq[:, s, :, 0:H],
                        axis=mybir.AxisListType.X,
                        op=mybir.AluOpType.add,
                    ).then_inc(cons, 1)
                elif kind == "P":
                    s = c % NSLOT_P
                    k = c // NSLOT_P + 1
                    v.wait_ge(gp_sem[s], 48 * k)
                    pview = hvp[:, s].rearrange("p (a b) k -> p a (b k)", b=2)
                    for h in range(2):
                        v.tensor_tensor(
                            out=pview[:, :, h * D : (h + 1) * D],
                            in0=hup[:, s, h],
                            in1=pview[:, :, h * D : (h + 1) * D],
                            op=mybir.AluOpType.mult,
                        )
                    v.tensor_tensor(
                        out=hvp[:, s, :, 0:H],
                        in0=hvp[:, s, :, 0:H],
                        in1=hvp[:, s, :, H:D],
                        op=mybir.AluOpType.add,
                    )
                    v.tensor_reduce(
                        out=osb[:, nqc * 16 + c * 16 : nqc * 16 + (c + 1) * 16],
                        in_=hvp[:, s, :, 0:H],
                        axis=mybir.AxisListType.X,
                        op=mybir.AluOpType.add,
                    ).then_inc(cons, 1)
                else:
                    s = c % NSLOT_S
                    k = c // NSLOT_S + 1
                    v.wait_ge(gs_sem[s], 32 * k)
                    v.tensor_tensor(
                        out=hvs[:, s],
                        in0=hus[:, s],
                        in1=hvs[:, s],
                        op=mybir.AluOpType.mult,
                    )
                    v.tensor_tensor(
                        out=hvs[:, s, :, 0:H],
                        in0=hvs[:, s, :, 0:H],
                        in1=hvs[:, s, :, H:D],
                        op=mybir.AluOpType.add,
                    )
                    base = (nqc + npc) * 16
                    v.tensor_reduce(
                        out=osb[:, base + c * 8 : base + (c + 1) * 8],
                        in_=hvs[:, s, :, 0:H],
                        axis=mybir.AxisListType.X,
                        op=mybir.AluOpType.add,
                    ).then_inc(cons, 1)

        @block.sync
        def _(sy):
            sy.dma_start(didxq_sb[:], didxq[:]).then_inc(io, 16)
            sy.dma_start(sidxq_sb[:], sidxq[:]).then_inc(io, 16)
            sy.dma_start(didxp_sb[:], didxp[:]).then_inc(io, 16)
            sy.dma_start(sidxp_sb[:], sidxp[:]).then_inc(io, 16)
            sy.dma_start(sidxs_sb[:], sidxs[:]).then_inc(io, 16)
            sy.dma_start(didxs_sb[:], didxs[:]).then_inc(io, 16)
            sy.wait_ge(cons, nqc + npc + nsc)
            sy.dma_start(y[:, :], osb[:, :]).then_inc(io2, 16)
            sy.wait_ge(io2, 16)

    nc.compile()
    return nc


def _split4(d_loc):
    """Greedy segmentation of the dense non-decreasing row sequence into
    quads (4 consecutive rows), pairs (2) and singles. Returns start-
    position arrays; members of a segment are consecutive positions."""
    n = len(d_loc)
    quads, pairs, singles = [], [], []
    d = d_loc
    i = 0
    while i < n:
        if (i + 3 < n and d[i + 1] == d[i] + 1 and d[i + 2] == d[i] + 2
                and d[i + 3] == d[i] + 3):
            quads.append(i)
            i += 4
        elif i + 1 < n and d[i + 1] == d[i] + 1:
            pairs.append(i)
            i += 2
        else:
            singles.append(i)
            i += 1
    return (np.array(quads, np.int64), np.array(pairs, np.int64),
            np.array(singles, np.int64))


def _prep_core(s_j, d_loc, ids_j, qs, ps, ss, nqc, npc, nsc):
    """Build wrapped idx tensors + (eid, ycol, ypart) mapping for one core."""
    nq_pad = max(1, nqc) * QCHUNK
    np_pad = max(1, npc) * CHUNK_E
    ns_pad = max(1, nsc) * CHUNK_E

    # quads: member j of quad at start position g is edge position g+j
    qu = np.zeros((4, nq_pad), np.int16)
    qv = np.zeros(nq_pad, np.int16)
    qe = np.full((4, nq_pad), -1, np.int64)
    nq = len(qs)
    for j in range(4):
        qu[j, :nq] = s_j[qs + j].astype(np.int16)
        qe[j, :nq] = ids_j[qs + j]
    qv[:nq] = d_loc[qs].astype(np.int16)
    for c in range(nqc):
        sl = slice(c * QCHUNK, (c + 1) * QCHUNK)
        perm = np.argsort(qu[0, sl], kind="stable")
        qu[:, sl] = qu[:, sl][:, perm]
        qv[sl] = qv[sl][perm]
        qe[:, sl] = qe[:, sl][:, perm]

    pu = np.zeros((2, np_pad), np.int16)
    pv = np.zeros(np_pad, np.int16)
    pe = np.full((2, np_pad), -1, np.int64)
    npr = len(ps)
    for j in range(2):
        pu[j, :npr] = s_j[ps + j].astype(np.int16)
        pe[j, :npr] = ids_j[ps + j]
    pv[:npr] = d_loc[ps].astype(np.int16)
    for c in range(npc):
        sl = slice(c * CHUNK_E, (c + 1) * CHUNK_E)
        perm = np.argsort(pu[0, sl], kind="stable")
        pu[:, sl] = pu[:, sl][:, perm]
        pv[sl] = pv[sl][perm]
        pe[:, sl] = pe[:, sl][:, perm]

    su = np.zeros(ns_pad, np.int16)
    sv = np.zeros(ns_pad, np.int16)
    se = np.full(ns_pad, -1, np.int64)
    nsg = len(ss)
    su[:nsg] = s_j[ss].astype(np.int16)
    sv[:nsg] = d_loc[ss].astype(np.int16)
    se[:nsg] = ids_j[ss]
    for c in range(nsc):
        sl = slice(c * CHUNK_E, (c + 1) * CHUNK_E)
        perm = np.argsort(su[sl], kind="stable")
        su[sl] = su[sl][perm]
        sv[sl] = sv[sl][perm]
        se[sl] = se[sl][perm]

    # quad hu idx: call h position (t*128+p) holds member (8h+t)%4 of quad
    # ((8h+t)//4)*128+p  ->  hv row 8h+t after the gather
    qsrc = np.zeros(max(1, nqc) * 2 * CHUNK_E, np.int16)
    for c in range(nqc):
        for h in range(2):
            for t in range(8):
                r = 8 * h + t
                a, j = r // 4, r % 4
                dstsl = (2 * c + h) * CHUNK_E + t * 128
                qsrc[dstsl : dstsl + 128] = qu[j, c * QCHUNK + a * 128 :
                                               c * QCHUNK + a * 128 + 128]
    # pair hu idx: call h covers half h of each pair (interleaved rows)
    psrc = pu.T.reshape(max(1, npc), CHUNK_E, 2).transpose(0, 2, 1).reshape(-1)

    sidxq = _wrap_idx(qsrc, max(1, nqc) * 2, CHUNK_E)
    didxq = _wrap_idx(qv, max(1, nqc), QCHUNK)
    sidxp = _wrap_idx(psrc, max(1, npc) * 2, CHUNK_E)
    didxp = _wrap_idx(pv, max(1, npc), CHUNK_E)
    sidxs = _wrap_idx(su, max(1, nsc), CHUNK_E)
    didxs = _wrap_idx(sv, max(1, nsc), CHUNK_E)

    eids, cols, parts = [], [], []
    for c in range(nqc):
        for r in range(16):
            a, j = r // 4, r % 4
            eids.append(qe[j, c * QCHUNK + a * 128 : c * QCHUNK + a * 128 + 128])
            cols.append(np.full(128, c * 16 + r))
            parts.append(np.arange(128))
    for c in range(npc):
        idx = np.arange(c * CHUNK_E, (c + 1) * CHUNK_E)
        b = (idx - c * CHUNK_E) // 128
        for h in range(2):
            eids.append(pe[h, idx])
            cols.append(nqc * 16 + c * 16 + 2 * b + h)
            parts.append(idx % 128)
    for c in range(nsc):
        idx = np.arange(c * CHUNK_E, (c + 1) * CHUNK_E)
        b = (idx - c * CHUNK_E) // 128
        eids.append(se[idx])
        cols.append((nqc + npc) * 16 + c * 8 + b)
        parts.append(idx % 128)
    return (sidxq, didxq, sidxp, didxp, sidxs, didxs,
            np.concatenate(eids), np.concatenate(cols), np.concatenate(parts))


def kernel(ufeat, ifeat, src, dst):
    from concourse.bass_utils import run_bass_kernel_spmd

    ufeat_h = np.ascontiguousarray(np.asarray(ufeat, dtype=np.float32)).astype(F16)
    ifeat_h = np.ascontiguousarray(np.asarray(ifeat, dtype=np.float32)).astype(F16)
    src_f = np.asarray(src).ravel().astype(np.int64)
    dst_f = np.asarray(dst).ravel().astype(np.int64)
    assert src_f.shape == (E,) and dst_f.shape == (E,)

    cores = []
    for j in range(NCORES):
        lo, hi = j * ECORE, (j + 1) * ECORE
        d_j = dst_f[lo:hi]
        order = np.argsort(d_j, kind="stable")
        uniq, d_loc = np.unique(d_j[order], return_inverse=True)
        s_j = src_f[lo:hi][order]
        ids_j = np.arange(lo, hi)[order]
        qs, ps, ss = _split4(d_loc)
        cores.append((s_j, d_loc, uniq, ids_j, qs, ps, ss))

    vcap = max(len(u) for (_, _, u, _, _, _, _) in cores)
    nqc = max(_cdiv(len(qs), QCHUNK) for (_, _, _, _, qs, _, _) in cores)
    npc = max(_cdiv(len(ps), CHUNK_E) for (_, _, _, _, _, ps, _) in cores)
    nsc = max(_cdiv(len(ss), CHUNK_E) for (_, _, _, _, _, _, ss) in cores)

    key = (nqc, npc, nsc, vcap)
    if key not in _PROGRAM_CACHE:
        _PROGRAM_CACHE[key] = _build_program(nqc, npc, nsc, vcap)
    nc = _PROGRAM_CACHE[key]

    in_maps = []
    maps = []
    for j in range(NCORES):
        s_j, d_loc, uniq, ids_j, qs, ps, ss = cores[j]
        vtab = np.zeros((vcap + 3, D), F16)
        vtab[: len(uniq)] = ifeat_h[uniq]
        sidxq, didxq, sidxp, didxp, sidxs, didxs, eid, ycol, ypart = _prep_core(
            s_j, d_loc, ids_j, qs, ps, ss, nqc, npc, nsc)
        in_maps.append({"ufeat": ufeat_h, "vtab": vtab,
                        "sidxq": sidxq, "didxq": didxq,
                        "sidxp": sidxp, "didxp": didxp,
                        "sidxs": sidxs, "didxs": didxs})
        maps.append((eid, ycol, ypart))

    res = run_bass_kernel_spmd(nc, in_maps, core_ids=list(range(NCORES)))

    out = np.empty((E, 1), np.float32)
    for j in range(NCORES):
        yj = np.asarray(res.results[j]["y"])   # [128, ycols]
        eid, ycol, ypart = maps[j]
        m = eid >= 0
        out[eid[m], 0] = yj[ypart[m], ycol[m]]
    return out
